# revision 27
# baseline (speedup 1.0000x reference)
"""Trainium2 Bass kernel for nn_BceLogitsLossWeighted.

Reference computation (N=8, L=8192, C=3):
  1. Weighted BCE-with-logits, mean over (N,L), weighted sum over C.
  2. "Proximity" between channels 1 and 2 per batch element, built from a
     full cross-correlation.  mean(full_conv(s1, rev s2)) == sum(s1)*sum(s2)/(2L-1)
     exactly, so the whole convolution collapses to sum/sum-of-squares reductions.
  3. Sparsity terms over channels 1..2 of input and target.

Everything reduces to per-(n, c) sums of elementwise quantities:
  bce identity:  log_sigmoid(x) = x - softplus(x), log_sigmoid(-x) = -softplus(x)
    => bce = -pw*t*x + ((pw-1)*t + 1) * softplus(x)
    => sum_l bce = (pw-1)*S_tsp + S_sp - pw*S_tx
       with S_tsp = sum t*softplus(x), S_sp = sum softplus(x), S_tx = sum t*x
  proximity:     needs S_x, S_x2 for channels 1, 2 of input
  sparsity:      d_l = x_l - x_0;  sum(|d| - 0.5*max(d,0)) = 0.75*Sabs - 0.25*Sd
       since max(d,0) = (|d|+d)/2:  |d| - 0.5*max(d,0) = 0.75*|d| - 0.25*d
  and S_x = Sd + L*x_0 comes free from the sparsity partials.

Sharding: data-parallel over batch N -> 8 cores, one batch element each.
Each core emits per-partition partial sums ([128, 19]); the host does the
final tiny O(N*C) combine in float64.

Engine split per core:
  ACT : one manual LoadActFuncSet(natural_log_exp_and_others) issued at t0 so
        the ~1.3us table load hides behind the input DMAs; then exp and
        ln(1+e) for softplus -- zero further table loads.
  DVE : fused scalar_tensor_tensor product-reduces + channel-major
        tensor_reduce (with apply_absolute_value for sum|d|).
  DMA : X load on sync/HWDGE, T load on Pool/SWDGE, single output on sync.

The x/t first-row values needed for the sparsity diffs are pre-broadcast by
the host into 6 extra columns of the X transfer ([128, 64*3 + 6]), so they
arrive with the main data and no on-device partition broadcast is needed.
"""

import numpy as np

import concourse.bass as bass
import concourse.tile as tile
from concourse import bacc, mybir
from concourse.bass_utils import run_bass_kernel_spmd

N, L, C = 8, 8192, 3
P = 128          # SBUF partitions
A = L // P       # 64 rows of l per partition
F = A * C        # 192 data columns per partition
XW = F + 2 * C   # X transfer width: data + broadcast x0/t0 columns
F32 = mybir.dt.float32

SPARSITY_A = 0.1
SPARSITY_B = 0.5

ACT_TABLE_NATURAL_LOG_EXP = 6  # index into get_activation_tables("gen3")

TRACE = False
LAST_RESULTS = None

_built = None


def _build():
    op = mybir.AluOpType
    act = mybir.ActivationFunctionType

    nc = bacc.Bacc(
        "TRN2", target_bir_lowering=False, debug=False, num_devices=N
    )
    x_d = nc.dram_tensor("x", [P, XW], F32, kind="ExternalInput")
    t_d = nc.dram_tensor("t", [P, A, C], F32, kind="ExternalInput")
    pv_d = nc.dram_tensor("pv", [P, 19], F32, kind="ExternalOutput")

    with tile.TileContext(nc) as tc:
        with (
            tc.tile_pool(name="main", bufs=1) as main,
            tc.tile_pool(name="scr", bufs=8) as scr,
        ):
            # Load the one ACT table set (exp + ln) up front; the
            # insert_act_table_loads fixpoint then sees it resident and adds
            # no further loads, and the ~1.3us load hides behind the DMAs.
            nc.scalar.add_instruction(
                mybir.InstLoadActFuncSet(
                    name=nc.get_next_instruction_name(), ins=[], outs=[],
                    act_func_set_id=ACT_TABLE_NATURAL_LOG_EXP,
                )
            )

            # --- loads on independent queues ---
            XB = main.tile([P, XW], F32)
            T = main.tile([P, A, C], F32)
            nc.sync.dma_start(out=XB, in_=x_d[:, :])
            nc.gpsimd.dma_start(out=T, in_=t_d[:, :, :])

            X = XB[:, 0:F].rearrange("p (a c) -> p a c", c=C)
            bc = XB[:, F:XW]   # [P, 6]: x0 c0..2, t0 c0..2

            # memset on DVE: runs in DVE's pre-data idle window and keeps the
            # D-sub STTs' dependency same-engine (no extra wait split)
            ZERO = main.tile([P, A], F32)
            nc.vector.memset(ZERO, 0.0)

            # --- softplus(x) = ln(1 + exp(x)) on ACT, flat 2D APs ---
            E = main.tile([P, F], F32)
            SP = main.tile([P, F], F32)
            nc.scalar.activation(out=E, in_=XB[:, 0:F], func=act.Exp)
            nc.scalar.activation(out=SP, in_=E, func=act.Ln, bias=1.0)
            SPv = SP.rearrange("p (a c) -> p a c", c=C)

            PV = main.tile([P, 19], F32)

            def stt(in0, in1, col, op0, op1, scalar=0.0, out=None):
                if out is None:
                    out = scr.tile([P, A], F32, tag="scr")
                nc.vector.scalar_tensor_tensor(
                    out=out, in0=in0, scalar=scalar, in1=in1,
                    op0=op0, op1=op1, accum_out=PV[:, col : col + 1],
                )

            # --- ACT: S_x2 via Square+accum (same table set), after ln ---
            for i, c in enumerate((1, 2)):
                s = scr.tile([P, A], F32, tag="ascr")
                nc.scalar.activation(out=s, in_=X[:, :, c], func=act.Square,
                                     accum_out=PV[:, 6 + i : 7 + i])

            # sparsity diffs d = v - v0, channel-major tiles; accum gives S_d
            Dx = main.tile([P, 2, A], F32)
            Dt = main.tile([P, 2, A], F32)
            for i, c in enumerate((1, 2)):
                stt(X[:, :, c], ZERO, 15 + i, op.subtract, op.add,
                    scalar=bc[:, c : c + 1], out=Dx[:, i, :])

            # --- T-dependent ---
            for c in range(C):
                stt(T[:, :, c], X[:, :, c], 3 + c, op.add, op.mult)   # S_tx
            for i, c in enumerate((1, 2)):
                stt(T[:, :, c], ZERO, 17 + i, op.subtract, op.add,
                    scalar=bc[:, C + c : C + c + 1], out=Dt[:, i, :])

            # --- softplus-dependent products ---
            for c in range(C):
                stt(T[:, :, c], SPv[:, :, c], c, op.add, op.mult)     # S_tsp

            # --- channel-major reduces last (scheduler packs STTs tighter) ---
            nc.vector.tensor_reduce(out=PV[:, 11:13], in_=Dx,
                axis=mybir.AxisListType.X, op=op.add, apply_absolute_value=True)
            nc.vector.tensor_reduce(out=PV[:, 13:15], in_=Dt,
                axis=mybir.AxisListType.X, op=op.add, apply_absolute_value=True)
            nc.vector.tensor_reduce(out=PV[:, 8:11],
                in_=SP.rearrange("p (a c) -> p c a", c=C),
                axis=mybir.AxisListType.X, op=op.add)                 # S_sp

            nc.sync.dma_start(out=pv_d[:, :], in_=PV)

    nc.compile()
    return nc


def kernel(input, target, class_weights, pos_weight):
    global _built, LAST_RESULTS
    if _built is None:
        _built = _build()
    nc = _built

    input = np.ascontiguousarray(input, dtype=np.float32)
    target = np.ascontiguousarray(target, dtype=np.float32)

    in_maps = []
    for n in range(N):
        xb = np.empty((P, XW), dtype=np.float32)
        xb[:, 0:F] = input[n].reshape(P, F)
        xb[:, F : F + C] = input[n, 0, :]     # x0 broadcast to all partitions
        xb[:, F + C : XW] = target[n, 0, :]   # t0 broadcast
        in_maps.append({"x": xb, "t": target[n].reshape(P, A, C)})

    res = run_bass_kernel_spmd(nc, in_maps, core_ids=list(range(N)), trace=TRACE)
    LAST_RESULTS = res

    cw = np.asarray(class_weights, dtype=np.float64)
    pw = np.asarray(pos_weight, dtype=np.float64)

    loss = 0.0
    bce_sum = np.zeros(C, dtype=np.float64)
    for n in range(N):
        pv = res.results[n]["pv"].astype(np.float64)

        s_tsp = pv[:, 0:3].sum(axis=0)       # sum t * softplus(x), per c
        s_tx = pv[:, 3:6].sum(axis=0)        # sum t * x, per c
        s_x2 = pv[:, 6:8].sum(axis=0)        # sum x^2, c = 1, 2
        s_sp = pv[:, 8:11].sum(axis=0)       # sum softplus(x), per c

        bce_sum += (pw - 1.0) * s_tsp + s_sp - pw * s_tx

        abs_in = pv[:, 11:13].sum(axis=0)    # sum |d| input, c = 1, 2
        abs_tg = pv[:, 13:15].sum(axis=0)    # sum |d| target
        d_in = pv[:, 15:17].sum(axis=0)      # sum d input
        d_tg = pv[:, 17:19].sum(axis=0)      # sum d target

        # proximity(s1=input[n,:,1], s2=input[n,:,2])
        x0 = input[n, 0, 1:3].astype(np.float64)
        s_x = d_in + L * x0                  # sum x, c = 1, 2
        e1, e2 = s_x2[0], s_x2[1]
        norm = np.sqrt(e1 * e2)
        mean_cc = s_x[0] * s_x[1] / (2.0 * L - 1.0)
        loss += mean_cc / norm + 1.0 - np.sqrt(e1 + e2) / norm

        # sparsity: sum(|d| - 0.5*max(d,0)) = 0.75*sum|d| - 0.25*sum d
        s_in = SPARSITY_A * (0.75 * abs_in - 0.25 * d_in)
        s_tg = SPARSITY_A * (0.75 * abs_tg - 0.25 * d_tg)
        loss += np.sum(np.abs(s_in - s_tg) / (s_tg + 1.0))

    loss += np.sum(10.0 * cw * bce_sum / (N * L))
    return np.float32(loss)


# revision 30
# speedup vs baseline: 1.0082x; 1.0082x over previous
"""Trainium2 Bass kernel for nn_BceLogitsLossWeighted.

Reference computation (N=8, L=8192, C=3):
  1. Weighted BCE-with-logits, mean over (N,L), weighted sum over C.
  2. "Proximity" between channels 1 and 2 per batch element, built from a
     full cross-correlation.  mean(full_conv(s1, rev s2)) == sum(s1)*sum(s2)/(2L-1)
     exactly, so the whole convolution collapses to sum/sum-of-squares reductions.
  3. Sparsity terms over channels 1..2 of input and target.

Everything reduces to per-(n, c) sums of elementwise quantities:
  bce identity:  log_sigmoid(x) = x - softplus(x), log_sigmoid(-x) = -softplus(x)
    => bce = -pw*t*x + ((pw-1)*t + 1) * softplus(x)
    => sum_l bce = (pw-1)*S_tsp + S_sp - pw*S_tx
       with S_tsp = sum t*softplus(x), S_sp = sum softplus(x), S_tx = sum t*x
  proximity:     needs S_x, S_x2 for channels 1, 2 of input
  sparsity:      d_l = x_l - x_0;  sum(|d| - 0.5*max(d,0)) = 0.75*Sabs - 0.25*Sd
       since max(d,0) = (|d|+d)/2:  |d| - 0.5*max(d,0) = 0.75*|d| - 0.25*d
  and S_x = Sd + L*x_0 comes free from the sparsity partials.

Sharding: data-parallel over batch N -> 8 cores, one batch element each.
Each core emits per-partition partial sums ([128, 19]); the host does the
final tiny O(N*C) combine in float64.

Engine split per core:
  ACT : one manual LoadActFuncSet(natural_log_exp_and_others) issued at t0 so
        the ~1.3us table load hides behind the input DMAs; then exp and
        ln(1+e) for softplus -- zero further table loads.
  DVE : fused scalar_tensor_tensor product-reduces + channel-major
        tensor_reduce (with apply_absolute_value for sum|d|).
  DMA : X load on sync/HWDGE, T load on Pool/SWDGE, single output on sync.

The x/t first-row values needed for the sparsity diffs are pre-broadcast by
the host into 6 extra columns of the X transfer ([128, 64*3 + 6]), so they
arrive with the main data and no on-device partition broadcast is needed.
"""

import numpy as np

import concourse.bass as bass
import concourse.tile as tile
from concourse import bacc, mybir
from concourse.bass_utils import run_bass_kernel_spmd

N, L, C = 8, 8192, 3
P = 128          # SBUF partitions
A = L // P       # 64 rows of l per partition
F = A * C        # 192 data columns per partition
XW = F + 2 * C   # X transfer width: data + broadcast x0/t0 columns
F32 = mybir.dt.float32

SPARSITY_A = 0.1
SPARSITY_B = 0.5

ACT_TABLE_NATURAL_LOG_EXP = 6  # index into get_activation_tables("gen3")

TRACE = False
LAST_RESULTS = None

_built = None


def _build():
    op = mybir.AluOpType
    act = mybir.ActivationFunctionType

    nc = bacc.Bacc(
        "TRN2", target_bir_lowering=False, debug=False, num_devices=N
    )
    x_d = nc.dram_tensor("x", [P, XW], F32, kind="ExternalInput")
    t_d = nc.dram_tensor("t", [P, A, C], F32, kind="ExternalInput")
    pv_d = nc.dram_tensor("pv", [P, 19], F32, kind="ExternalOutput")

    with tile.TileContext(nc) as tc:
        with (
            tc.tile_pool(name="main", bufs=1) as main,
            tc.tile_pool(name="scr", bufs=8) as scr,
        ):
            # Load the one ACT table set (exp + ln) up front; the
            # insert_act_table_loads fixpoint then sees it resident and adds
            # no further loads, and the ~1.3us load hides behind the DMAs.
            nc.scalar.add_instruction(
                mybir.InstLoadActFuncSet(
                    name=nc.get_next_instruction_name(), ins=[], outs=[],
                    act_func_set_id=ACT_TABLE_NATURAL_LOG_EXP,
                )
            )

            # --- loads on independent queues ---
            XB = main.tile([P, XW], F32)
            T = main.tile([P, A, C], F32)
            nc.sync.dma_start(out=XB, in_=x_d[:, :])
            nc.gpsimd.dma_start(out=T, in_=t_d[:, :, :])

            X = XB[:, 0:F].rearrange("p (a c) -> p a c", c=C)
            bc = XB[:, F:XW]   # [P, 6]: x0 c0..2, t0 c0..2

            # memset on DVE: runs in DVE's pre-data idle window and keeps the
            # D-sub STTs' dependency same-engine (no extra wait split)
            ZERO = main.tile([P, A], F32)
            nc.vector.memset(ZERO, 0.0)

            # --- softplus(x) = ln(1 + exp(x)) on ACT, flat 2D APs ---
            PV = main.tile([P, 19], F32)

            E = main.tile([P, F], F32)
            SP = main.tile([P, F], F32)
            nc.scalar.activation(out=E, in_=XB[:, 0:F], func=act.Exp)
            # accum_out gives sum(softplus) over ALL channels for free; the
            # host recovers channel 2 as M - S_sp0 - S_sp1
            nc.scalar.activation(out=SP, in_=E, func=act.Ln, bias=1.0,
                                 accum_out=PV[:, 10:11])
            SPv = SP.rearrange("p (a c) -> p a c", c=C)

            def stt(in0, in1, col, op0, op1, scalar=0.0, out=None):
                if out is None:
                    out = scr.tile([P, A], F32, tag="scr")
                nc.vector.scalar_tensor_tensor(
                    out=out, in0=in0, scalar=scalar, in1=in1,
                    op0=op0, op1=op1, accum_out=PV[:, col : col + 1],
                )

            # --- ACT: S_x2 via Square+accum (same table set), after ln ---
            for i, c in enumerate((1, 2)):
                s = scr.tile([P, A], F32, tag="ascr")
                nc.scalar.activation(out=s, in_=X[:, :, c], func=act.Square,
                                     accum_out=PV[:, 6 + i : 7 + i])

            # sparsity diffs d = v - v0, channel-major tiles; accum gives S_d
            Dx = main.tile([P, 2, A], F32)
            Dt = main.tile([P, 2, A], F32)
            for i, c in enumerate((1, 2)):
                stt(X[:, :, c], ZERO, 15 + i, op.subtract, op.add,
                    scalar=bc[:, c : c + 1], out=Dx[:, i, :])

            # --- T-dependent ---
            for c in range(C):
                stt(T[:, :, c], X[:, :, c], 3 + c, op.add, op.mult)   # S_tx
            for i, c in enumerate((1, 2)):
                stt(T[:, :, c], ZERO, 17 + i, op.subtract, op.add,
                    scalar=bc[:, C + c : C + c + 1], out=Dt[:, i, :])

            # --- softplus-dependent products ---
            for c in range(C):
                stt(T[:, :, c], SPv[:, :, c], c, op.add, op.mult)     # S_tsp

            # --- channel-major reduces last (scheduler packs STTs tighter) ---
            nc.vector.tensor_reduce(out=PV[:, 11:13], in_=Dx,
                axis=mybir.AxisListType.X, op=op.add, apply_absolute_value=True)
            nc.vector.tensor_reduce(out=PV[:, 13:15], in_=Dt,
                axis=mybir.AxisListType.X, op=op.add, apply_absolute_value=True)
            nc.vector.tensor_reduce(out=PV[:, 8:10],
                in_=SP.rearrange("p (a c) -> p c a", c=C)[:, 0:2, :],
                axis=mybir.AxisListType.X, op=op.add)                 # S_sp c0,c1

            nc.sync.dma_start(out=pv_d[:, :], in_=PV)

    nc.compile()
    return nc


def kernel(input, target, class_weights, pos_weight):
    global _built, LAST_RESULTS
    if _built is None:
        _built = _build()
    nc = _built

    input = np.ascontiguousarray(input, dtype=np.float32)
    target = np.ascontiguousarray(target, dtype=np.float32)

    in_maps = []
    for n in range(N):
        xb = np.empty((P, XW), dtype=np.float32)
        xb[:, 0:F] = input[n].reshape(P, F)
        xb[:, F : F + C] = input[n, 0, :]     # x0 broadcast to all partitions
        xb[:, F + C : XW] = target[n, 0, :]   # t0 broadcast
        in_maps.append({"x": xb, "t": target[n].reshape(P, A, C)})

    res = run_bass_kernel_spmd(nc, in_maps, core_ids=list(range(N)), trace=TRACE)
    LAST_RESULTS = res

    cw = np.asarray(class_weights, dtype=np.float64)
    pw = np.asarray(pos_weight, dtype=np.float64)

    loss = 0.0
    bce_sum = np.zeros(C, dtype=np.float64)
    for n in range(N):
        pv = res.results[n]["pv"].astype(np.float64)

        s_tsp = pv[:, 0:3].sum(axis=0)       # sum t * softplus(x), per c
        s_tx = pv[:, 3:6].sum(axis=0)        # sum t * x, per c
        s_x2 = pv[:, 6:8].sum(axis=0)        # sum x^2, c = 1, 2
        sp01 = pv[:, 8:10].sum(axis=0)       # sum softplus(x), c = 0, 1
        sp_all = pv[:, 10].sum(axis=0)       # sum softplus(x), all channels
        s_sp = np.array([sp01[0], sp01[1], sp_all - sp01[0] - sp01[1]])

        bce_sum += (pw - 1.0) * s_tsp + s_sp - pw * s_tx

        abs_in = pv[:, 11:13].sum(axis=0)    # sum |d| input, c = 1, 2
        abs_tg = pv[:, 13:15].sum(axis=0)    # sum |d| target
        d_in = pv[:, 15:17].sum(axis=0)      # sum d input
        d_tg = pv[:, 17:19].sum(axis=0)      # sum d target

        # proximity(s1=input[n,:,1], s2=input[n,:,2])
        x0 = input[n, 0, 1:3].astype(np.float64)
        s_x = d_in + L * x0                  # sum x, c = 1, 2
        e1, e2 = s_x2[0], s_x2[1]
        norm = np.sqrt(e1 * e2)
        mean_cc = s_x[0] * s_x[1] / (2.0 * L - 1.0)
        loss += mean_cc / norm + 1.0 - np.sqrt(e1 + e2) / norm

        # sparsity: sum(|d| - 0.5*max(d,0)) = 0.75*sum|d| - 0.25*sum d
        s_in = SPARSITY_A * (0.75 * abs_in - 0.25 * d_in)
        s_tg = SPARSITY_A * (0.75 * abs_tg - 0.25 * d_tg)
        loss += np.sum(np.abs(s_in - s_tg) / (s_tg + 1.0))

    loss += np.sum(10.0 * cw * bce_sum / (N * L))
    return np.float32(loss)
